# revision 15
# baseline (speedup 1.0000x reference)
"""DeepWDK fused single-launch Trainium2 kernel.

Sequence order is permuted so core c owns block c = [X1[64c:64c+64], X2[64c:64c+64]].
Feature order of V is f = d*20 + a (d-major) so per-seq V^T tiles load
contiguously. Pair-matmul contraction uses k-order (b, l) with l padded to 256
so all psum->pair-layout reshuffles are single affine DMAs. One-hot operands
travel as fp8e4m3 (exact for 0/1); value operands as bf16.

Single NEFF per core, phases:
 1. VtP[f, s] partial = T'^T @ O (own 512-slice of the 4096 contraction) ->
    bf16 -> ReduceScatter(add): core c gets Vt_own [1280, 128] for its seqs.
 2. 128 per-seq PE matmuls S[n] = Vt[n].T @ Vt[n] -> s_bf on SBUF.
 3. 128 per-seq gather matmuls A1^T[i] = S1[i] @ O1[i]^T (B2 same form) ->
    [20, 2, 128, 64] bf16 -> DRAM bounce -> k-major [128, 40, 64] tiles.
 4. k1/k2 self-kernels = diag(U1val^T @ U1oh) via PE + identity-mask reduce.
 5. Two AllGathers (O2 one-hot fp8, B2 values bf16) -> pair matmul
    K_block[64, 512] = U1_own^T @ U2_all (80 k-chunks) -> f32 to host.
Host: tiny T' einsum + one-hot scatters in, K normalization out.
"""
import numpy as np
import ml_dtypes

import concourse.mybir as mybir
import concourse.tile as tile
from concourse import bacc
from concourse.bass_utils import run_bass_kernel_spmd
from concourse.masks import make_identity

BF = ml_dtypes.bfloat16
F8 = ml_dtypes.float8_e4m3
F32 = mybir.dt.float32
BF16 = mybir.dt.bfloat16
FP8 = mybir.dt.float8e4
INT8 = mybir.dt.int8

N_AA = 20
D = 64
E_DIM = 32
L = 200
LP = 256                  # l padded (so each aa row-block = 2 k-chunks)
N1 = 512
N2 = 512
NCORES = 8
NSEQ = N1 + N2            # 1024
MDIM = N_AA * D           # 1280
KA = L * N_AA             # 4000
KA_PAD = 4096
KA_CORE = KA_PAD // NCORES   # 512
BLK = NSEQ // NCORES         # 128 seqs per core (64 X1 + 64 X2)
HB = BLK // 2                # 64
KV = N_AA * LP               # 5120 = values-half k
KVC = KV // 128              # 40 chunks
KPC = 2 * KVC                # 80 chunks total

RG = [list(range(NCORES))]

ONE_BF = np.uint16(0x3F80)   # bf16 1.0 bit pattern
ONE_F8 = np.uint8(0x38)      # fp8e4m3 1.0 bit pattern


def _build_fused():
    nc = bacc.Bacc("TRN2", target_bir_lowering=False, debug=False,
                   num_devices=NCORES)
    Tt = nc.dram_tensor("Tt", [4, 128, MDIM], BF16, kind="ExternalInput")
    Ot = nc.dram_tensor("Ot", [4, 128, NSEQ], FP8, kind="ExternalInput")
    Xo1 = nc.dram_tensor("Xo1", [1, 2, 128, HB], INT8, kind="ExternalInput")
    Xo2 = nc.dram_tensor("Xo2", [1, 2, 128, HB], INT8, kind="ExternalInput")
    AVEC = nc.dram_tensor("AVEC", [N_AA, 1], F32, kind="ExternalInput")
    Kout = nc.dram_tensor("Kout", [HB, N2], BF16, kind="ExternalOutput")
    KD = nc.dram_tensor("KD", [HB, 2], F32, kind="ExternalOutput")

    with tile.TileContext(nc) as tc:
        with tc.tile_pool(name="const", bufs=1) as cpool, \
             tc.tile_pool(name="out", bufs=4) as opool, \
             tc.tile_pool(name="dram", bufs=1, space="DRAM") as dpool, \
             tc.tile_pool(name="ps1p", bufs=2, space="PSUM") as ps1pool, \
             tc.tile_pool(name="ps2p", bufs=1, space="PSUM") as ps2pool, \
             tc.tile_pool(name="ps3p", bufs=2, space="PSUM") as ps3pool, \
             tc.tile_pool(name="pskp", bufs=1, space="PSUM") as pskpool, \
             tc.tile_pool(name="psdp", bufs=2, space="PSUM") as psdpool, \
             tc.tile_pool(name="u2ld", bufs=6) as u2pool:

            # ---------------- Phase 1: Vt partial + ReduceScatter ----------
            tts, ots = [], []
            for kc in range(4):
                t = cpool.tile([128, MDIM], BF16, tag=f"tt{kc}")
                nc.sync.dma_start(out=t, in_=Tt[kc])
                tts.append(t)
                o = cpool.tile([128, NSEQ], FP8, tag=f"ot{kc}")
                nc.sync.dma_start(out=o, in_=Ot[kc])
                ots.append(o)

            vt_part = dpool.tile([NCORES, MDIM, BLK], BF16)
            vt_own = dpool.tile([MDIM, BLK], BF16)

            for fc in range(10):
                for h in range(2):
                    ps = ps1pool.tile([128, 512], F32, tag="ps1")
                    for kc in range(4):
                        nc.tensor.matmul(
                            ps,
                            tts[kc][:, fc * 128:(fc + 1) * 128],
                            ots[kc][:, h * 512:(h + 1) * 512],
                            start=(kc == 0), stop=(kc == 3),
                        )
                    ob = opool.tile([128, 512], BF16, tag="ob1")
                    nc.vector.tensor_copy(out=ob, in_=ps)
                    for b in range(4):
                        nc.sync.dma_start(
                            out=vt_part[4 * h + b, fc * 128:(fc + 1) * 128, :],
                            in_=ob[:, b * 128:(b + 1) * 128],
                        )

            nc.gpsimd.collective_compute(
                "ReduceScatter",
                mybir.AluOpType.add,
                replica_groups=RG,
                ins=[vt_part.opt()],
                outs=[vt_own.opt()],
            )

            # ---------------- Phase 2: per-seq S --------------------------
            vt_sb = cpool.tile([D, N_AA, BLK], BF16, tag="vt_sb")
            nc.sync.dma_start(out=vt_sb,
                              in_=vt_own[:].rearrange("(d a) n -> d a n", d=D))

            s_bf = cpool.tile([N_AA, BLK * N_AA], BF16, tag="s_bf")
            for g in range(6):          # groups of 24 seqs
                n0 = g * 24
                cnt = min(24, BLK - n0)
                ps = ps2pool.tile([N_AA, 480], F32, tag="ps2")
                for j in range(cnt):
                    n = n0 + j
                    nc.tensor.matmul(
                        ps[:, j * N_AA:(j + 1) * N_AA],
                        vt_sb[:, :, n],
                        vt_sb[:, :, n],
                        start=True, stop=True,
                    )
                nc.vector.tensor_copy(
                    out=s_bf[:, n0 * N_AA:(n0 + cnt) * N_AA],
                    in_=ps[:, :cnt * N_AA])

            # ---------------- Phase 3: on-device one-hot expansion --------
            # X indices: xk{1,2}[h] = [128 p, 64 i] int8 (l = h*128+p; pad = -1)
            avec = cpool.tile([N_AA, 1], F32, tag="avec")
            nc.sync.dma_start(out=avec, in_=AVEC[:])
            xk1, xk2 = [], []
            for h in range(2):
                x1t = cpool.tile([128, HB], INT8, tag=f"xk1_{h}")
                nc.sync.dma_start(out=x1t, in_=Xo1[0, h])
                xk1.append(x1t)
                x2t = cpool.tile([128, HB], INT8, tag=f"xk2_{h}")
                nc.sync.dma_start(out=x2t, in_=Xo2[0, h])
                xk2.append(x2t)
            # X broadcast across the 20 aa partitions for the gather layout
            xb1 = cpool.tile([N_AA, 2, 128, HB], INT8, tag="xb1")
            xb2 = cpool.tile([N_AA, 2, 128, HB], INT8, tag="xb2")
            for a in range(N_AA):
                nc.sync.dma_start(out=xb1[a:a + 1], in_=Xo1[:])
                nc.sync.dma_start(out=xb2[a:a + 1], in_=Xo2[:])
            # per-seq one-hot in gather layout [20 a, 256 l, 64 i]
            o1g = cpool.tile([N_AA, LP, HB], FP8, tag="o1g")
            nc.vector.tensor_scalar(
                out=o1g, in0=xb1.rearrange("a h p i -> a (h p) i"),
                scalar1=avec, scalar2=None, op0=mybir.AluOpType.is_equal)
            o2g = cpool.tile([N_AA, LP, HB], FP8, tag="o2g")
            nc.vector.tensor_scalar(
                out=o2g, in0=xb2.rearrange("a h p i -> a (h p) i"),
                scalar1=avec, scalar2=None, op0=mybir.AluOpType.is_equal)

            a1sb = cpool.tile([N_AA, 2, 128, HB], BF16, tag="a1sb")
            b2sb = cpool.tile([N_AA, 2, 128, HB], BF16, tag="b2sb")
            nc.any.memset(a1sb, 0.0)
            nc.any.memset(b2sb, 0.0)
            for half, dest, og in ((0, a1sb, o1g), (1, b2sb, o2g)):
                for i in range(HB):
                    j = half * HB + i
                    ps = ps3pool.tile([N_AA, L], F32, tag="ps3")
                    nc.tensor.matmul(
                        ps,
                        s_bf[:, j * N_AA:(j + 1) * N_AA],
                        og[:, 0:L, i],
                        start=True, stop=True,
                    )
                    nc.any.tensor_copy(
                        out=dest[:, :, :, i].rearrange("b h p -> b (h p)")[:, :L],
                        in_=ps,
                    )

            # DRAM bounces (flat layout == k-major [KV, 64])
            a1dram = dpool.tile([N_AA, 2, 128, HB], BF16)
            nc.sync.dma_start(out=a1dram, in_=a1sb)
            b2dram = dpool.tile([N_AA, 2, 128, HB], BF16)
            nc.sync.dma_start(out=b2dram, in_=b2sb)

            # k-major SBUF tiles
            u1v = cpool.tile([128, KVC, HB], BF16, tag="u1v")
            nc.sync.dma_start(out=u1v,
                              in_=a1dram.rearrange("b h p i -> p (b h) i"))
            u1o = cpool.tile([128, KVC, HB], FP8, tag="u1o")
            for kc in range(KVC):
                nc.vector.tensor_scalar(
                    out=u1o[:, kc, :], in0=xk1[kc % 2], scalar1=float(kc // 2),
                    scalar2=None, op0=mybir.AluOpType.is_equal)
            u2vk = cpool.tile([128, KVC, HB], BF16, tag="u2vk")
            nc.sync.dma_start(out=u2vk,
                              in_=b2dram.rearrange("b h p i -> p (b h) i"))
            u2ok = cpool.tile([128, KVC, HB], FP8, tag="u2ok")
            for kc in range(KVC):
                nc.vector.tensor_scalar(
                    out=u2ok[:, kc, :], in0=xk2[kc % 2], scalar1=float(kc // 2),
                    scalar2=None, op0=mybir.AluOpType.is_equal)

            # ---------------- Phase 3.5: k1/k2 self-kernels ---------------
            ident = cpool.tile([HB, HB], F32, tag="ident")
            make_identity(nc, ident)
            kd_sb = cpool.tile([HB, 2], F32, tag="kd_sb")
            for col, lv, lo in ((0, u1v, u1o), (1, u2vk, u2ok)):
                psd = psdpool.tile([HB, HB], F32, tag="psd")
                for kc in range(KVC):
                    nc.tensor.matmul(
                        psd, lv[:, kc, :], lo[:, kc, :],
                        start=(kc == 0), stop=(kc == KVC - 1),
                    )
                prod = opool.tile([HB, HB], F32, tag="prod")
                nc.vector.tensor_mul(prod, psd, ident)
                nc.vector.tensor_reduce(
                    out=kd_sb[:, col:col + 1], in_=prod,
                    axis=mybir.AxisListType.X, op=mybir.AluOpType.add)
            nc.sync.dma_start(out=KD[:], in_=kd_sb)

            # ---------------- AllGathers ----------------------------------
            o2b = dpool.tile([KVC, 128, HB], FP8)
            nc.sync.dma_start(out=o2b[:].rearrange("kc p i -> p kc i"),
                              in_=u2ok)
            u2all_oh = dpool.tile([NCORES, KVC, 128, HB], FP8,
                                  addr_space="Shared")
            nc.gpsimd.collective_compute(
                "AllGather", mybir.AluOpType.bypass, replica_groups=RG,
                ins=[o2b.opt()], outs=[u2all_oh.opt()])
            u2all_val = dpool.tile([NCORES, N_AA, 2, 128, HB], BF16,
                                   addr_space="Shared")
            nc.gpsimd.collective_compute(
                "AllGather", mybir.AluOpType.bypass, replica_groups=RG,
                ins=[b2dram.opt()], outs=[u2all_val.opt()])

            # ---------------- Phase 4: pair matmul ------------------------
            psk = pskpool.tile([HB, N2], F32, tag="psk")
            for kc in range(KPC):
                if kc < KVC:
                    u2t = u2pool.tile([128, NCORES, HB], FP8, tag="u2t_oh")
                    nc.sync.dma_start(
                        out=u2t,
                        in_=u2all_oh[:, kc, :, :].rearrange("r p i -> p r i"))
                    lhsT = u1v[:, kc, :]
                else:
                    k2c = kc - KVC
                    u2t = u2pool.tile([128, NCORES, HB], BF16, tag="u2t_v")
                    nc.sync.dma_start(
                        out=u2t,
                        in_=u2all_val[:, k2c // 2, k2c % 2, :, :]
                            .rearrange("r p i -> p r i"))
                    lhsT = u1o[:, k2c, :]
                nc.tensor.matmul(
                    psk, lhsT, u2t,
                    start=(kc == 0), stop=(kc == KPC - 1),
                )
            obk = opool.tile([HB, N2], BF16, tag="obk")
            nc.vector.tensor_copy(out=obk, in_=psk)
            nc.sync.dma_start(out=Kout[:], in_=obk)
    nc.finalize()
    return nc


_FUSED_NC = None


def _get_fused():
    global _FUSED_NC
    if _FUSED_NC is None:
        _FUSED_NC = _build_fused()
    return _FUSED_NC


def _host_pre(X1, X2, E, W):
    W3 = W.reshape(L, E_DIM, MDIM)
    T = np.matmul(E[None], W3)                  # (200, 20, 1280)
    Tb = np.zeros((KA_PAD, MDIM), dtype=BF)
    Tb[:KA] = (T.astype(BF).reshape(KA, N_AA, D)
               .transpose(0, 2, 1).reshape(KA, MDIM))

    X1b = X1.reshape(NCORES, HB, L)
    X2b = X2.reshape(NCORES, HB, L)
    Xp = np.concatenate([X1b, X2b], axis=1).reshape(NSEQ, L)
    OTu = np.zeros((KA_PAD, NSEQ), dtype=np.uint8)
    rows = np.arange(L)[None, :] * N_AA + Xp
    OTu[rows, np.arange(NSEQ)[:, None]] = ONE_F8
    return Tb, OTu.view(F8), Xp, X1b, X2b


def _xpad_blk(Xblk):
    """Xblk (HB, L) -> padded int8 X^T [1, 2, 128, HB] (pad rows = -1)."""
    xo = np.full((LP, HB), -1, dtype=np.int8)
    xo[:L] = Xblk.T
    return xo.reshape(1, 2, 128, HB)


_AVEC = np.arange(N_AA, dtype=np.float32)[:, None]


def kernel(X1, X2, E, W, a):
    X1 = np.asarray(X1).astype(np.int64)
    X2 = np.asarray(X2).astype(np.int64)
    E = np.asarray(E, dtype=np.float32)
    W = np.asarray(W, dtype=np.float32)
    a = np.asarray(a, dtype=np.float32)

    Tb, OT, Xp, X1b, X2b = _host_pre(X1, X2, E, W)
    nc = _get_fused()
    cores = list(range(NCORES))

    in_maps = []
    for c in cores:
        in_maps.append({
            "Tt": Tb[c * KA_CORE:(c + 1) * KA_CORE].reshape(4, 128, MDIM),
            "Ot": OT[c * KA_CORE:(c + 1) * KA_CORE].reshape(4, 128, NSEQ),
            "Xo1": _xpad_blk(X1b[c]),
            "Xo2": _xpad_blk(X2b[c]),
            "AVEC": _AVEC,
        })

    res = run_bass_kernel_spmd(nc, in_maps, cores)

    k1 = np.concatenate([res.results[c]["KD"][:, 0] for c in cores])
    k2 = np.concatenate([res.results[c]["KD"][:, 1] for c in cores])
    Kmat = np.concatenate(
        [res.results[c]["Kout"].astype(np.float32) for c in cores], axis=0)
    Kmat = 0.5 * Kmat / np.sqrt(k1)[:, None] / np.sqrt(k2)[None, :]
    return (a.reshape(-1)[0] ** 2 * Kmat).astype(np.float32)


# revision 16
# speedup vs baseline: 1.0193x; 1.0193x over previous
"""DeepWDK fused single-launch Trainium2 kernel.

Sequence order is permuted so core c owns block c = [X1[64c:64c+64], X2[64c:64c+64]].
Feature order of V is f = d*20 + a (d-major) so per-seq V^T tiles load
contiguously. Pair-matmul contraction uses k-order (b, l) with l padded to 256
so all psum->pair-layout reshuffles are single affine DMAs. One-hot operands
travel as fp8e4m3 (exact for 0/1); value operands as bf16.

Single NEFF per core, phases:
 1. VtP[f, s] partial = T'^T @ O (own 512-slice of the 4096 contraction) ->
    bf16 -> ReduceScatter(add): core c gets Vt_own [1280, 128] for its seqs.
 2. 128 per-seq PE matmuls S[n] = Vt[n].T @ Vt[n] -> s_bf on SBUF.
 3. 128 per-seq gather matmuls A1^T[i] = S1[i] @ O1[i]^T (B2 same form) ->
    [20, 2, 128, 64] bf16 -> DRAM bounce -> k-major [128, 40, 64] tiles.
 4. k1/k2 self-kernels = diag(U1val^T @ U1oh) via PE + identity-mask reduce.
 5. Two AllGathers (O2 one-hot fp8, B2 values bf16) -> pair matmul
    K_block[64, 512] = U1_own^T @ U2_all (80 k-chunks) -> f32 to host.
Host: tiny T' einsum + one-hot scatters in, K normalization out.
"""
import numpy as np
import ml_dtypes

import concourse.mybir as mybir
import concourse.tile as tile
from concourse import bacc
from concourse.bass_utils import run_bass_kernel_spmd
from concourse.masks import make_identity

BF = ml_dtypes.bfloat16
F8 = ml_dtypes.float8_e4m3
F32 = mybir.dt.float32
BF16 = mybir.dt.bfloat16
FP8 = mybir.dt.float8e4
INT8 = mybir.dt.int8

N_AA = 20
D = 64
E_DIM = 32
L = 200
LP = 256                  # l padded (so each aa row-block = 2 k-chunks)
N1 = 512
N2 = 512
NCORES = 8
NSEQ = N1 + N2            # 1024
MDIM = N_AA * D           # 1280
KA = L * N_AA             # 4000
KA_PAD = 4096
KA_CORE = KA_PAD // NCORES   # 512
BLK = NSEQ // NCORES         # 128 seqs per core (64 X1 + 64 X2)
HB = BLK // 2                # 64
KV = N_AA * LP               # 5120 = values-half k
KVC = KV // 128              # 40 chunks
KPC = 2 * KVC                # 80 chunks total

RG = [list(range(NCORES))]

ONE_BF = np.uint16(0x3F80)   # bf16 1.0 bit pattern
ONE_F8 = np.uint8(0x38)      # fp8e4m3 1.0 bit pattern


def _build_fused():
    nc = bacc.Bacc("TRN2", target_bir_lowering=False, debug=False,
                   num_devices=NCORES)
    Tt = nc.dram_tensor("Tt", [4, 128, MDIM], BF16, kind="ExternalInput")
    Ot = nc.dram_tensor("Ot", [4, 128, NSEQ], FP8, kind="ExternalInput")
    Xo1 = nc.dram_tensor("Xo1", [1, 2, 128, HB], INT8, kind="ExternalInput")
    Xo2 = nc.dram_tensor("Xo2", [1, 2, 128, HB], INT8, kind="ExternalInput")
    AVEC = nc.dram_tensor("AVEC", [N_AA, 1], F32, kind="ExternalInput")
    Kout = nc.dram_tensor("Kout", [HB, N2], BF16, kind="ExternalOutput")
    KD = nc.dram_tensor("KD", [HB, 2], F32, kind="ExternalOutput")

    with tile.TileContext(nc) as tc:
        with tc.tile_pool(name="const", bufs=1) as cpool, \
             tc.tile_pool(name="out", bufs=4) as opool, \
             tc.tile_pool(name="dram", bufs=1, space="DRAM") as dpool, \
             tc.tile_pool(name="ps1p", bufs=2, space="PSUM") as ps1pool, \
             tc.tile_pool(name="ps2p", bufs=1, space="PSUM") as ps2pool, \
             tc.tile_pool(name="ps3p", bufs=2, space="PSUM") as ps3pool, \
             tc.tile_pool(name="pskp", bufs=1, space="PSUM") as pskpool, \
             tc.tile_pool(name="psdp", bufs=2, space="PSUM") as psdpool, \
             tc.tile_pool(name="u2ld", bufs=6) as u2pool:

            # ---------------- Phase 1: Vt partial + ReduceScatter ----------
            tts, ots = [], []
            for kc in range(4):
                t = cpool.tile([128, MDIM], BF16, tag=f"tt{kc}")
                nc.sync.dma_start(out=t, in_=Tt[kc])
                tts.append(t)
                o = cpool.tile([128, NSEQ], FP8, tag=f"ot{kc}")
                nc.sync.dma_start(out=o, in_=Ot[kc])
                ots.append(o)

            vt_part = dpool.tile([NCORES, MDIM, BLK], BF16)
            vt_own = dpool.tile([MDIM, BLK], BF16)

            for fc in range(10):
                for h in range(2):
                    ps = ps1pool.tile([128, 512], F32, tag="ps1")
                    for kc in range(4):
                        nc.tensor.matmul(
                            ps,
                            tts[kc][:, fc * 128:(fc + 1) * 128],
                            ots[kc][:, h * 512:(h + 1) * 512],
                            start=(kc == 0), stop=(kc == 3),
                        )
                    ob = opool.tile([128, 512], BF16, tag="ob1")
                    nc.vector.tensor_copy(out=ob, in_=ps)
                    for b in range(4):
                        nc.sync.dma_start(
                            out=vt_part[4 * h + b, fc * 128:(fc + 1) * 128, :],
                            in_=ob[:, b * 128:(b + 1) * 128],
                        )

            nc.gpsimd.collective_compute(
                "ReduceScatter",
                mybir.AluOpType.add,
                replica_groups=RG,
                ins=[vt_part.opt()],
                outs=[vt_own.opt()],
            )

            # ---------------- Phase 2: per-seq S --------------------------
            vt_sb = cpool.tile([D, N_AA, BLK], BF16, tag="vt_sb")
            nc.sync.dma_start(out=vt_sb,
                              in_=vt_own[:].rearrange("(d a) n -> d a n", d=D))

            s_bf = cpool.tile([N_AA, BLK * N_AA], BF16, tag="s_bf")
            for g in range(6):          # groups of 24 seqs
                n0 = g * 24
                cnt = min(24, BLK - n0)
                ps = ps2pool.tile([N_AA, 480], F32, tag="ps2")
                for j in range(cnt):
                    n = n0 + j
                    nc.tensor.matmul(
                        ps[:, j * N_AA:(j + 1) * N_AA],
                        vt_sb[:, :, n],
                        vt_sb[:, :, n],
                        start=True, stop=True,
                    )
                nc.vector.tensor_copy(
                    out=s_bf[:, n0 * N_AA:(n0 + cnt) * N_AA],
                    in_=ps[:, :cnt * N_AA])

            # ---------------- Phase 3: on-device one-hot expansion --------
            # X indices: xk{1,2}[h] = [128 p, 64 i] int8 (l = h*128+p; pad = -1)
            avec = cpool.tile([N_AA, 1], F32, tag="avec")
            nc.sync.dma_start(out=avec, in_=AVEC[:])
            xk1, xk2 = [], []
            for h in range(2):
                x1t = cpool.tile([128, HB], INT8, tag=f"xk1_{h}")
                nc.sync.dma_start(out=x1t, in_=Xo1[0, h])
                xk1.append(x1t)
                x2t = cpool.tile([128, HB], INT8, tag=f"xk2_{h}")
                nc.sync.dma_start(out=x2t, in_=Xo2[0, h])
                xk2.append(x2t)
            # X broadcast across the 20 aa partitions for the gather layout
            xb1 = cpool.tile([N_AA, 2, 128, HB], INT8, tag="xb1")
            xb2 = cpool.tile([N_AA, 2, 128, HB], INT8, tag="xb2")
            for a in range(N_AA):
                nc.sync.dma_start(out=xb1[a:a + 1], in_=Xo1[:])
                nc.sync.dma_start(out=xb2[a:a + 1], in_=Xo2[:])
            # per-seq one-hot in gather layout [20 a, 256 l, 64 i]
            o1g = cpool.tile([N_AA, LP, HB], FP8, tag="o1g")
            nc.vector.tensor_scalar(
                out=o1g, in0=xb1.rearrange("a h p i -> a (h p) i"),
                scalar1=avec, scalar2=None, op0=mybir.AluOpType.is_equal)
            o2g = cpool.tile([N_AA, LP, HB], FP8, tag="o2g")
            nc.vector.tensor_scalar(
                out=o2g, in0=xb2.rearrange("a h p i -> a (h p) i"),
                scalar1=avec, scalar2=None, op0=mybir.AluOpType.is_equal)

            a1sb = cpool.tile([N_AA, 2, 128, HB], BF16, tag="a1sb")
            b2sb = cpool.tile([N_AA, 2, 128, HB], BF16, tag="b2sb")
            nc.any.memset(a1sb, 0.0)
            nc.any.memset(b2sb, 0.0)
            for half, dest, og in ((0, a1sb, o1g), (1, b2sb, o2g)):
                for i in range(HB):
                    j = half * HB + i
                    ps = ps3pool.tile([N_AA, L], F32, tag="ps3")
                    nc.tensor.matmul(
                        ps,
                        s_bf[:, j * N_AA:(j + 1) * N_AA],
                        og[:, 0:L, i],
                        start=True, stop=True,
                    )
                    nc.any.tensor_copy(
                        out=dest[:, :, :, i].rearrange("b h p -> b (h p)")[:, :L],
                        in_=ps,
                    )

            # DRAM bounces (flat layout == k-major [KV, 64])
            a1dram = dpool.tile([N_AA, 2, 128, HB], BF16)
            nc.sync.dma_start(out=a1dram, in_=a1sb)
            pack = dpool.tile([2, N_AA, 2, 128, HB], BF16)

            # k-major SBUF tiles
            u1v = cpool.tile([128, KVC, HB], BF16, tag="u1v")
            nc.sync.dma_start(out=u1v,
                              in_=a1dram.rearrange("b h p i -> p (b h) i"))
            u1o = cpool.tile([128, KVC, HB], FP8, tag="u1o")
            for kc in range(KVC):
                nc.vector.tensor_scalar(
                    out=u1o[:, kc, :], in0=xk1[kc % 2], scalar1=float(kc // 2),
                    scalar2=None, op0=mybir.AluOpType.is_equal)
            u2vk = cpool.tile([128, KVC, HB], BF16, tag="u2vk")
            nc.sync.dma_start(out=u2vk,
                              in_=pack[1].rearrange("b h p i -> p (b h) i"))
            u2ok = cpool.tile([128, KVC, HB], BF16, tag="u2ok")
            for kc in range(KVC):
                nc.vector.tensor_scalar(
                    out=u2ok[:, kc, :], in0=xk2[kc % 2], scalar1=float(kc // 2),
                    scalar2=None, op0=mybir.AluOpType.is_equal)

            # ---------------- Phase 3.5: k1/k2 self-kernels ---------------
            ident = cpool.tile([HB, HB], F32, tag="ident")
            make_identity(nc, ident)
            kd_sb = cpool.tile([HB, 2], F32, tag="kd_sb")
            for col, lv, lo in ((0, u1v, u1o), (1, u2vk, u2ok)):
                psd = psdpool.tile([HB, HB], F32, tag="psd")
                for kc in range(KVC):
                    nc.tensor.matmul(
                        psd, lv[:, kc, :], lo[:, kc, :],
                        start=(kc == 0), stop=(kc == KVC - 1),
                    )
                prod = opool.tile([HB, HB], F32, tag="prod")
                nc.vector.tensor_mul(prod, psd, ident)
                nc.vector.tensor_reduce(
                    out=kd_sb[:, col:col + 1], in_=prod,
                    axis=mybir.AxisListType.X, op=mybir.AluOpType.add)
            nc.sync.dma_start(out=KD[:], in_=kd_sb)

            # ---------------- single packed AllGather ---------------------
            nc.sync.dma_start(
                out=pack[0].rearrange("b h p i -> p (b h) i"), in_=u2ok)
            nc.sync.dma_start(out=pack[1], in_=b2sb)
            u2all = dpool.tile([NCORES, 2, N_AA, 2, 128, HB], BF16,
                               addr_space="Shared")
            nc.gpsimd.collective_compute(
                "AllGather", mybir.AluOpType.bypass, replica_groups=RG,
                ins=[pack.opt()], outs=[u2all.opt()])

            # ---------------- Phase 4: pair matmul ------------------------
            psk = pskpool.tile([HB, N2], F32, tag="psk")
            for kc in range(KPC):
                half, kh = (0, kc) if kc < KVC else (1, kc - KVC)
                u2t = u2pool.tile([128, NCORES, HB], BF16, tag="u2t")
                nc.sync.dma_start(
                    out=u2t,
                    in_=u2all[:, half, kh // 2, kh % 2, :, :]
                        .rearrange("r p i -> p r i"))
                lhsT = u1v[:, kh, :] if half == 0 else u1o[:, kh, :]
                nc.tensor.matmul(
                    psk, lhsT, u2t,
                    start=(kc == 0), stop=(kc == KPC - 1),
                )
            obk = opool.tile([HB, N2], BF16, tag="obk")
            nc.vector.tensor_copy(out=obk, in_=psk)
            nc.sync.dma_start(out=Kout[:], in_=obk)
    nc.finalize()
    return nc


_FUSED_NC = None


def _get_fused():
    global _FUSED_NC
    if _FUSED_NC is None:
        _FUSED_NC = _build_fused()
    return _FUSED_NC


def _host_pre(X1, X2, E, W):
    W3 = W.reshape(L, E_DIM, MDIM)
    T = np.matmul(E[None], W3)                  # (200, 20, 1280)
    Tb = np.zeros((KA_PAD, MDIM), dtype=BF)
    Tb[:KA] = (T.astype(BF).reshape(KA, N_AA, D)
               .transpose(0, 2, 1).reshape(KA, MDIM))

    X1b = X1.reshape(NCORES, HB, L)
    X2b = X2.reshape(NCORES, HB, L)
    Xp = np.concatenate([X1b, X2b], axis=1).reshape(NSEQ, L)
    OTu = np.zeros((KA_PAD, NSEQ), dtype=np.uint8)
    rows = np.arange(L)[None, :] * N_AA + Xp
    OTu[rows, np.arange(NSEQ)[:, None]] = ONE_F8
    return Tb, OTu.view(F8), Xp, X1b, X2b


def _xpad_blk(Xblk):
    """Xblk (HB, L) -> padded int8 X^T [1, 2, 128, HB] (pad rows = -1)."""
    xo = np.full((LP, HB), -1, dtype=np.int8)
    xo[:L] = Xblk.T
    return xo.reshape(1, 2, 128, HB)


_AVEC = np.arange(N_AA, dtype=np.float32)[:, None]


def kernel(X1, X2, E, W, a):
    X1 = np.asarray(X1).astype(np.int64)
    X2 = np.asarray(X2).astype(np.int64)
    E = np.asarray(E, dtype=np.float32)
    W = np.asarray(W, dtype=np.float32)
    a = np.asarray(a, dtype=np.float32)

    Tb, OT, Xp, X1b, X2b = _host_pre(X1, X2, E, W)
    nc = _get_fused()
    cores = list(range(NCORES))

    in_maps = []
    for c in cores:
        in_maps.append({
            "Tt": Tb[c * KA_CORE:(c + 1) * KA_CORE].reshape(4, 128, MDIM),
            "Ot": OT[c * KA_CORE:(c + 1) * KA_CORE].reshape(4, 128, NSEQ),
            "Xo1": _xpad_blk(X1b[c]),
            "Xo2": _xpad_blk(X2b[c]),
            "AVEC": _AVEC,
        })

    res = run_bass_kernel_spmd(nc, in_maps, cores)

    k1 = np.concatenate([res.results[c]["KD"][:, 0] for c in cores])
    k2 = np.concatenate([res.results[c]["KD"][:, 1] for c in cores])
    Kmat = np.concatenate(
        [res.results[c]["Kout"].astype(np.float32) for c in cores], axis=0)
    Kmat = 0.5 * Kmat / np.sqrt(k1)[:, None] / np.sqrt(k2)[None, :]
    return (a.reshape(-1)[0] ** 2 * Kmat).astype(np.float32)
